# revision 1
# baseline (speedup 1.0000x reference)
"""CentroidInstanceLoss on 8 Trainium2 NeuronCores (Bass/Tile).

Data-parallel over points: each of the 8 cores processes N/8 = 32768 points.
Per-core segment sums (via one-hot matmuls) are combined with a
ReduceScatter; the [512, 257] centroid(+pull-weight) table is AllGathered
back; a second pass over the points computes the pull term; the push term
uses partition-rotated centroid diffs on the core owning each subbatch.
Host does only O(S*L) label bookkeeping and the final ~70-float combine.
"""

import numpy as np

import concourse.bass as bass
import concourse.bacc as bacc
import concourse.mybir as mybir
import concourse.tile as tile

f32 = mybir.dt.float32
f16 = mybir.dt.float16
HALF = True
fdat = f16 if HALF else f32

# Problem shape (hardcoded per contract).
N_TOTAL = 262144
D = 256
S = 8
L = 64
NSEG = S * L  # 512
NCORES = 8
DELTA_V = 0.5
DELTA_D = 1.5

AluOp = mybir.AluOpType
ActFn = mybir.ActivationFunctionType


def build_nc(n_core: int, use_collectives: bool = True, reps: int = 1,
             phases: tuple = ("p1", "cc", "push", "p2")):
    """Build the SPMD Bass program for one core holding n_core points.

    use_collectives=False builds a single-core variant (collectives replaced
    with local DMA) for TimelineSim profiling. reps>1 replicates the body for
    marginal-time measurement on hardware.
    """
    assert n_core % 128 == 0
    T = n_core // 128  # point tiles per core
    G = min(8, T)      # norm-batch group size
    assert T % G == 0

    nc = bacc.Bacc(
        "TRN2", target_bir_lowering=False, debug=False,
        num_devices=NCORES if use_collectives else 1,
    )

    x_in = nc.dram_tensor("x", [n_core, D], fdat, kind="ExternalInput")
    segrow_in = nc.dram_tensor("segrow", [n_core], fdat, kind="ExternalInput")
    segcol_in = nc.dram_tensor("segcol", [128, T], f32, kind="ExternalInput")
    sbcol_in = nc.dram_tensor("sbcol", [128, T], f32, kind="ExternalInput")
    iota512_in = nc.dram_tensor("iota512", [128, NSEG], fdat, kind="ExternalInput")
    iotapc_in = nc.dram_tensor("iotapc", [128, 4], f32, kind="ExternalInput")
    iota8_in = nc.dram_tensor("iota8", [128, S], f32, kind="ExternalInput")
    ones_in = nc.dram_tensor("ones1", [1, 128], fdat, kind="ExternalInput")
    perms_in = nc.dram_tensor("perms", [L, L - 1, L], fdat, kind="ExternalInput")
    wblk_in = nc.dram_tensor("wblk", [L, 1], f32, kind="ExternalInput")
    crecip_in = nc.dram_tensor("crecip", [L, 1], f32, kind="ExternalInput")

    lpull_out = nc.dram_tensor("lpull", [S, 1], f32, kind="ExternalOutput")
    qrot_out = nc.dram_tensor("qrot", [L, L], f32, kind="ExternalOutput")

    segrow_v = segrow_in.ap().rearrange("(t i) -> t i", i=128)  # [T, 128]

    with tile.TileContext(nc) as tc:
        with (
            tc.tile_pool(name="const", bufs=1) as constp,
            tc.tile_pool(name="norm", bufs=1) as normp,
            tc.tile_pool(name="mu", bufs=1) as mup,
            tc.tile_pool(name="dram", bufs=1, space="DRAM") as dram,
            tc.tile_pool(name="x1", bufs=4) as xp1,
            tc.tile_pool(name="oh", bufs=4) as ohp,
            tc.tile_pool(name="sqc", bufs=2) as sqcp,
        ):
            # ---- constants ----
            iota512_sb = constp.tile([128, NSEG], fdat)
            nc.sync.dma_start(iota512_sb[:], iota512_in[:])
            iotapc_sb = constp.tile([128, 4], f32)
            nc.sync.dma_start(iotapc_sb[:], iotapc_in[:])
            iota8_sb = constp.tile([128, S], f32)
            nc.sync.dma_start(iota8_sb[:], iota8_in[:])
            ones_sb = constp.tile([1, 128], fdat)
            nc.sync.dma_start(ones_sb[:], ones_in[:])
            segcol_sb = constp.tile([128, T], f32)
            nc.sync.dma_start(segcol_sb[:], segcol_in[:])
            sbcol_sb = constp.tile([128, T], f32)
            nc.sync.dma_start(sbcol_sb[:], sbcol_in[:])
            wblk_sb = constp.tile([L, 1], f32)
            nc.sync.dma_start(wblk_sb[:], wblk_in[:])
            crecip_sb = constp.tile([L, 1], f32)
            nc.sync.dma_start(crecip_sb[:], crecip_in[:])
            perms_sb = constp.tile([L, L - 1, L], fdat)
            nc.sync.dma_start(perms_sb[:], perms_in[:])
            negdv_sb = constp.tile([128, 1], f32)
            nc.vector.memset(negdv_sb[:], -DELTA_V)

            for rep in range(reps):
                ss_all = normp.tile([128, T], f32, tag="ss", name="ss_all")
                rr_all = normp.tile([128, T], f32, tag="rr", name="rr_all")

                # ---- pass 1: per-core segment sums of normalized points ----
                with tc.tile_pool(name="psum1", bufs=1, space="PSUM") as psum1:
                    ps_sums = [
                        psum1.tile([128, D], f32, tag=f"sums{c}", name=f"ps_sums{c}")
                        for c in range(4)
                    ]
                    for g in range(T // G if "p1" in phases else 0):
                        t0 = g * G
                        xb = xp1.tile([128, G, D], fdat, tag="x1t")
                        nc.sync.dma_start(
                            xb[:],
                            x_in[t0 * 128:(t0 + G) * 128, :].rearrange(
                                "(g p) d -> p g d", p=128),
                        )
                        for j in range(G):
                            t = t0 + j
                            sink = sqcp.tile([128, D], fdat, tag="sq_sink")
                            if j % 2 == 0:
                                nc.vector.scalar_tensor_tensor(
                                    sink[:], xb[:, j, :], 1.0, xb[:, j, :],
                                    op0=AluOp.bypass, op1=AluOp.mult,
                                    accum_out=ss_all[:, t:t + 1],
                                )
                            else:
                                nc.scalar.activation(
                                    sink[:], xb[:, j, :], ActFn.Square,
                                    accum_out=ss_all[:, t:t + 1],
                                )
                        sqc = sqcp.tile([128, G], f32, tag="sqc")
                        nc.scalar.activation(
                            sqc[:], ss_all[:, g * G:(g + 1) * G], ActFn.Sqrt
                        )
                        nc.vector.tensor_scalar_add(sqc[:], sqc[:], 1e-8)
                        nc.vector.reciprocal(rr_all[:, g * G:(g + 1) * G], sqc[:])
                        for j in range(G):
                            t = t0 + j
                            oh = ohp.tile([128, NSEG], fdat, tag="oh")
                            nc.gpsimd.tensor_scalar(
                                oh[:, 0:384], iota512_sb[:, 0:384],
                                segcol_sb[:, t:t + 1], rr_all[:, t:t + 1],
                                op0=AluOp.is_equal, op1=AluOp.mult,
                            )
                            nc.vector.tensor_scalar(
                                oh[:, 384:NSEG], iota512_sb[:, 384:NSEG],
                                segcol_sb[:, t:t + 1], rr_all[:, t:t + 1],
                                op0=AluOp.is_equal, op1=AluOp.mult,
                            )
                            for c in range(4):
                                nc.tensor.matmul(
                                    ps_sums[c][:],
                                    oh[:, c * 128:(c + 1) * 128],
                                    xb[:, j, :],
                                    start=(t == 0), stop=(t == T - 1),
                                )

                    rs_in = dram.tile([NSEG, D], f32, tag="rs_in", name="rs_in")
                    for c in range(4):
                        sums_sb = sqcp.tile(
                            [128, D], f32, tag="sums_sb", name="sums_sb"
                        )
                        nc.vector.tensor_copy(sums_sb[:], ps_sums[c][:])
                        nc.sync.dma_start(
                            rs_in[c * 128:(c + 1) * 128, :], sums_sb[:]
                        )

                # ---- combine centroid table across cores ----
                rs_out = dram.tile([L, D], f32, tag="rs_out", name="rs_out")
                if "cc" not in phases:
                    nc.sync.dma_start(rs_out[:], rs_in[0:L, :])
                elif use_collectives:
                    nc.gpsimd.collective_compute(
                        "ReduceScatter", AluOp.add,
                        replica_groups=[list(range(NCORES))],
                        ins=[rs_in.opt()], outs=[rs_out.opt()],
                    )
                else:
                    nc.sync.dma_start(rs_out[:], rs_in[0:L, :])
                musb_raw = mup.tile([L, D], f32, tag="musb", name="musb_raw")
                nc.sync.dma_start(musb_raw[:], rs_out[:])
                muaug = mup.tile([L, D + 1], f32, tag="muaug", name="muaug")
                nc.vector.tensor_scalar(
                    muaug[:, 0:D], musb_raw[:], crecip_sb[:, 0:1], None,
                    op0=AluOp.mult,
                )
                nc.vector.tensor_copy(muaug[:, D:D + 1], wblk_sb[:])
                ag_in = dram.tile([L, D + 1], f32, tag="ag_in", name="ag_in")
                nc.sync.dma_start(ag_in[:], muaug[:])
                ag_out = dram.tile(
                    [NSEG, D + 1], f32, tag="ag_out", name="ag_out",
                    addr_space="Shared" if use_collectives else "Local",
                )
                if use_collectives and "cc" in phases:
                    nc.gpsimd.collective_compute(
                        "AllGather", AluOp.bypass,
                        replica_groups=[list(range(NCORES))],
                        ins=[ag_in.opt()], outs=[ag_out.opt()],
                    )
                else:
                    for c in range(S):
                        nc.sync.dma_start(
                            ag_out[c * L:(c + 1) * L, :], ag_in[:]
                        )
                mut_sb = mup.tile([128, 4, D + 1], f32, tag="mut", name="mut_sb")
                nc.sync.dma_start(
                    mut_sb[:], ag_out.rearrange("(c p) d -> p c d", p=128)
                )
                mut_h = mup.tile([128, 4, D + 1], fdat, tag="muth", name="mut_h")
                nc.vector.tensor_copy(mut_h[:], mut_sb[:])

                # ---- push: pairwise centroid L1 distances (own subbatch) ----
                q_sb = mup.tile([L, L], f32, tag="q", name="q_sb")
                nc.vector.memset(q_sb[:, 0:1], 0.0)
                mua_h = mup.tile([L, D], fdat, tag="muah", name="mua_h")
                nc.vector.tensor_copy(mua_h[:], muaug[:, 0:D])
                with (
                    tc.tile_pool(name="rotps", bufs=2, space="PSUM") as rotpsp,
                    tc.tile_pool(name="pdiff", bufs=3) as pdp,
                ):
                    for k in range(1, L if "push" in phases else 1):
                        ps_rot = rotpsp.tile([L, D], f32, tag="rotps")
                        nc.tensor.matmul(
                            ps_rot[:], perms_sb[:, k - 1, :], mua_h[:],
                            start=True, stop=True,
                        )
                        pdiff = pdp.tile([L, D], f32, tag="pdiff")
                        nc.vector.tensor_sub(pdiff[:], mua_h[:], ps_rot[:])
                        psink = pdp.tile([L, D], f32, tag="psink")
                        nc.scalar.activation(
                            psink[:], pdiff[:], ActFn.Abs,
                            accum_out=q_sb[:, k:k + 1],
                        )
                nc.sync.dma_start(qrot_out[:], q_sb[:])

                # ---- pass 2: pull term ----
                with (
                    tc.tile_pool(name="x2", bufs=6) as xp2,
                    tc.tile_pool(name="srow", bufs=4) as srowp,
                    tc.tile_pool(name="bcps", bufs=3, space="PSUM") as bcpsp,
                    tc.tile_pool(name="bcsb", bufs=3) as bcsbp,
                    tc.tile_pool(name="oht", bufs=4) as ohtp,
                    tc.tile_pool(name="mups", bufs=3, space="PSUM") as mupsp,
                    tc.tile_pool(name="pullps", bufs=1, space="PSUM") as pullpsp,
                    tc.tile_pool(name="diff", bufs=3) as diffp,
                    tc.tile_pool(name="sink2", bufs=2) as sink2p,
                    tc.tile_pool(name="small", bufs=4) as smallp,
                ):
                    ps_pull = pullpsp.tile([S, 1], f32, tag="pull", name="ps_pull")
                    if "p2" not in phases:
                        nc.vector.memset(ps_pull[:], 0.0)
                    for t in range(T if "p2" in phases else 0):
                        j = t % G
                        if j == 0:
                            xb2 = xp2.tile([128, G, D], fdat, tag="x2t")
                            nc.sync.dma_start(
                                xb2[:],
                                x_in[t * 128:(t + G) * 128, :].rearrange(
                                    "(g p) d -> p g d", p=128),
                            )
                            srow8 = srowp.tile([1, G * 128], fdat, tag="srow")
                            nc.sync.dma_start(
                                srow8[:],
                                segrow_in.ap()[t * 128:(t + G) * 128]
                                .rearrange("(a i) -> a i", a=1),
                            )
                        xt = xb2[:, j, :]
                        srow = srow8[:, j * 128:(j + 1) * 128]
                        ps_bc = bcpsp.tile([128, 128], f32, tag="bc")
                        nc.tensor.matmul(
                            ps_bc[:], ones_sb[:], srow[:], start=True, stop=True
                        )
                        bc_sb = bcsbp.tile([128, 128], fdat, tag="bcsb")
                        nc.vector.tensor_copy(bc_sb[:], ps_bc[:])
                        oht = ohtp.tile([128, NSEG], fdat, tag="oht")
                        for c in range(4):
                            nc.gpsimd.tensor_scalar(
                                oht[:, c * 128:(c + 1) * 128], bc_sb[:],
                                iotapc_sb[:, c:c + 1], None,
                                op0=AluOp.is_equal,
                            )
                        ps_mu = mupsp.tile([128, D + 1], f32, tag="mu")
                        for c in range(4):
                            nc.tensor.matmul(
                                ps_mu[:],
                                oht[:, c * 128:(c + 1) * 128],
                                mut_h[:, c, :],
                                start=(c == 0), stop=(c == 3),
                            )
                        diff = diffp.tile([128, D], f32, tag="diff")
                        nc.vector.scalar_tensor_tensor(
                            diff[:], xt, rr_all[:, t:t + 1], ps_mu[:, 0:D],
                            op0=AluOp.mult, op1=AluOp.subtract,
                        )
                        sink = sink2p.tile([128, D], f32, tag="sink2")
                        d1 = smallp.tile([128, 1], f32, tag="d1")
                        nc.scalar.activation(
                            sink[:], diff[:], ActFn.Abs, accum_out=d1[:]
                        )
                        t1 = smallp.tile([128, 1], f32, tag="t1")
                        nc.scalar.activation(
                            t1[:], d1[:], ActFn.Relu, bias=negdv_sb[:]
                        )
                        t2 = smallp.tile([128, 1], f32, tag="t2")
                        nc.vector.tensor_mul(t2[:], t1[:], t1[:])
                        v = smallp.tile([128, 1], f32, tag="v")
                        nc.vector.tensor_mul(v[:], t2[:], ps_mu[:, D:D + 1])
                        ohsb = smallp.tile([128, S], f32, tag="ohsb")
                        nc.vector.tensor_scalar(
                            ohsb[:], iota8_sb[:], sbcol_sb[:, t:t + 1], None,
                            op0=AluOp.is_equal,
                        )
                        nc.tensor.matmul(
                            ps_pull[:], ohsb[:], v[:],
                            start=(t == 0), stop=(t == T - 1),
                        )
                    lpull_sb = smallp.tile([S, 1], f32, tag="lpull_sb")
                    nc.vector.tensor_copy(lpull_sb[:], ps_pull[:])
                    nc.sync.dma_start(lpull_out[:], lpull_sb[:])

    nc.compile()
    return nc


def host_tables(labels: np.ndarray, subbatch: np.ndarray):
    """Everything derivable from the integer inputs alone."""
    seg = (subbatch.astype(np.int64) * L + labels.astype(np.int64)).astype(np.int32)
    counts = np.bincount(seg, minlength=NSEG).astype(np.float64)  # [512]
    present = counts > 0
    M = present.reshape(S, L).sum(axis=1).astype(np.float64)  # [S]
    valid = M > 1.0
    # per-seg pull weight: valid(sb)/(M_sb * count_s); 0 for invalid sb
    M_per_seg = np.repeat(M, L)
    valid_per_seg = np.repeat(valid, L)
    w = np.where(
        valid_per_seg, 1.0 / (M_per_seg * np.maximum(counts, 1.0)), 0.0
    ).astype(np.float32)
    crecip = (1.0 / np.maximum(counts, 1.0)).astype(np.float32)
    return seg, counts, present, M, valid, w, crecip


def make_in_maps(outputs: np.ndarray, labels: np.ndarray, subbatch: np.ndarray):
    n = outputs.shape[0]
    n_core = n // NCORES
    T = n_core // 128
    seg, counts, present, M, valid, w, crecip = host_tables(labels, subbatch)
    segf = seg.astype(np.float32)
    sbf = subbatch.astype(np.float32)

    iota512 = np.broadcast_to(
        np.arange(NSEG, dtype=np.float32), (128, NSEG)
    ).copy()
    iotapc = (
        np.arange(4, dtype=np.float32)[None, :] * 128.0
        + np.arange(128, dtype=np.float32)[:, None]
    ).copy()  # [128, 4]
    iota8 = np.broadcast_to(np.arange(S, dtype=np.float32), (128, S)).copy()
    ones1 = np.ones((1, 128), dtype=np.float32)
    pp, kk, mm = np.meshgrid(
        np.arange(L), np.arange(1, L), np.arange(L), indexing="ij")
    perms = (pp == (mm + kk) % L).astype(np.float32)  # [L, L-1, L]

    in_maps = []
    for c in range(NCORES):
        sl = slice(c * n_core, (c + 1) * n_core)
        segc = segf[sl]
        sbc = sbf[sl]
        blk = slice(c * L, (c + 1) * L)
        in_maps.append({
            "x": np.ascontiguousarray(outputs[sl]).astype(np.float16) if HALF else np.ascontiguousarray(outputs[sl]),
            "segrow": segc.astype(np.float16) if HALF else segc,
            "segcol": np.ascontiguousarray(segc.reshape(T, 128).T),
            "sbcol": np.ascontiguousarray(sbc.reshape(T, 128).T),
            "iota512": iota512.astype(np.float16) if HALF else iota512,
            "iotapc": iotapc,
            "iota8": iota8,
            "ones1": ones1.astype(np.float16) if HALF else ones1,
            "perms": perms.astype(np.float16) if HALF else perms,
            "wblk": w[blk].reshape(L, 1),
            "crecip": crecip[blk].reshape(L, 1),
        })
    return in_maps, (seg, counts, present, M, valid, w, crecip)


def combine(results, tables, n: int):
    """Host combine of the per-core outputs into the scalar loss."""
    seg, counts, present, M, valid, w, crecip = tables
    pull_total = np.float64(0.0)
    for r in results:
        pull_total += r["lpull"].astype(np.float64).sum()

    push_total = np.float64(0.0)
    pres_sl = present.reshape(S, L)
    for sb in range(S):
        if not valid[sb]:
            continue
        q = results[sb]["qrot"].astype(np.float64)  # [64(a), 64(k)]
        a = np.arange(L)
        dist = np.zeros((L, L))
        for k in range(1, L):
            dist[a, (a + k) % L] = q[:, k]
        p = pres_sl[sb]
        mask = p[:, None] & p[None, :] & ~np.eye(L, dtype=bool)
        r = np.maximum(2.0 * DELTA_D - dist, 0.0) ** 2
        push = np.where(mask, r, 0.0).sum()
        push_total += push / max(M[sb] * (M[sb] - 1.0), 1.0)

    return np.float32((pull_total + push_total) / n)


_NC_CACHE: dict = {}


def _get_nc(n_core: int):
    if n_core not in _NC_CACHE:
        _NC_CACHE[n_core] = build_nc(n_core)
    return _NC_CACHE[n_core]


def kernel(outputs, labels, subbatch_indices):
    from concourse.bass_utils import run_bass_kernel_spmd

    outputs = np.asarray(outputs, dtype=np.float32)
    labels = np.asarray(labels, dtype=np.int32)
    subbatch_indices = np.asarray(subbatch_indices, dtype=np.int32)
    n = outputs.shape[0]
    n_core = n // NCORES

    nc = _get_nc(n_core)
    in_maps, tables = make_in_maps(outputs, labels, subbatch_indices)
    res = run_bass_kernel_spmd(nc, in_maps, list(range(NCORES)))
    return combine(res.results, tables, n)



# revision 3
# speedup vs baseline: 5.2199x; 5.2199x over previous
"""CentroidInstanceLoss on 8 Trainium2 NeuronCores (Bass/Tile).

Subbatch-parallel: core c processes exactly the points of subbatch c
(S=8 == NCORES), padded to a common tile count T_pad. All centroid
segment-sums are then core-LOCAL (64 labels per core): no collectives,
no cross-core barrier. Host does only integer bookkeeping (counts, M,
pull weights, the subbatch partition/pad permutation) and the final
O(S*L) combine.

Per core:
  pass 1: x resident in SBUF; per 128-point tile: sum-of-squares accum
          (ACT), rsqrt per group, one-hot [128,64]*rr (DVE), one matmul
          accumulating the [64, 256] centroid sums in PSUM.
  mu:     scale by 1/counts, append w*1024 column -> [128, 257] f16
          table (rows 64..127 zero).
  push:   63 partition-rotation matmuls on the [64,256] mu block.
  pass 2: per tile: seg broadcast matmul, one-hot transpose compare
          (DVE, from PSUM), one gather matmul -> per-point mu+w, diff,
          L1 accum; per group of G tiles: relu/square/weight + a
          ones-matmul partition reduction into per-tile pull sums.
Outputs: q [64,64] rotation distances, ts [1, T_pad] weighted pull
sums. Host: loss = (sum(ts)/WSCALE + push)/N.
"""

import numpy as np

import concourse.bass as bass
import concourse.bacc as bacc
import concourse.mybir as mybir
import concourse.tile as tile

f32 = mybir.dt.float32
f16 = mybir.dt.float16

# Problem shape (hardcoded per contract).
N_TOTAL = 262144
D = 256
S = 8
L = 64
NCORES = 8
DELTA_V = 0.5
DELTA_D = 1.5
G = 16          # tiles per group
WSCALE = 1024.0  # pull-weight scale to keep w in f16 normal range

AluOp = mybir.AluOpType
ActFn = mybir.ActivationFunctionType


def build_nc(T_pad: int, reps: int = 1,
             phases: tuple = ("p1", "push", "p2")):
    """SPMD program for one core holding T_pad tiles of 128 points."""
    assert T_pad % G == 0
    NCH = 8                      # x load chunks
    CHT = (T_pad + NCH - 1) // NCH

    nc = bacc.Bacc("TRN2", target_bir_lowering=False, debug=False,
                   num_devices=1)

    x_in = nc.dram_tensor("x", [128, T_pad * D], f16, kind="ExternalInput")
    segrow_in = nc.dram_tensor("segrow", [T_pad * 128], f16,
                               kind="ExternalInput")
    segcol_in = nc.dram_tensor("segcol", [128, T_pad], f32,
                               kind="ExternalInput")
    iota64_in = nc.dram_tensor("iota64", [128, L], f16, kind="ExternalInput")
    iotap_in = nc.dram_tensor("iotap", [128, 1], f32, kind="ExternalInput")
    ones_in = nc.dram_tensor("ones1", [1, 128], f16, kind="ExternalInput")
    onesw_in = nc.dram_tensor("onesw", [128, 1], f32, kind="ExternalInput")
    perms_in = nc.dram_tensor("perms", [L, (L - 1) * L], f16,
                              kind="ExternalInput")
    wblk_in = nc.dram_tensor("wblk", [L, 1], f32, kind="ExternalInput")
    crecip_in = nc.dram_tensor("crecip", [L, 1], f32, kind="ExternalInput")

    q_out = nc.dram_tensor("q", [L, L], f32, kind="ExternalOutput")
    ts_out = nc.dram_tensor("ts", [1, T_pad], f32, kind="ExternalOutput")

    with tile.TileContext(nc) as tc:
        with (
            tc.tile_pool(name="const", bufs=1) as constp,
            tc.tile_pool(name="xres", bufs=NCH) as xp,
            tc.tile_pool(name="norm", bufs=1) as normp,
            tc.tile_pool(name="oh", bufs=4) as ohp,
            tc.tile_pool(name="srow", bufs=3) as srowp,
            tc.tile_pool(name="oht", bufs=4) as ohtp,
            tc.tile_pool(name="diff", bufs=4) as diffp,
            tc.tile_pool(name="sink", bufs=3) as sinkp,
            tc.tile_pool(name="mut", bufs=1) as mutp,
            tc.tile_pool(name="grp", bufs=2) as grpp,
            tc.tile_pool(name="small", bufs=2) as smallp,
        ):
            # ---- constants ----
            iota64_sb = constp.tile([128, L], f16)
            nc.sync.dma_start(iota64_sb[:], iota64_in[:])
            iotap_sb = constp.tile([128, 1], f32)
            nc.sync.dma_start(iotap_sb[:], iotap_in[:])
            ones_sb = constp.tile([1, 128], f16)
            nc.sync.dma_start(ones_sb[:], ones_in[:])
            onesw_sb = constp.tile([128, 1], f32)
            nc.sync.dma_start(onesw_sb[:], onesw_in[:])
            segcol_sb = constp.tile([128, T_pad], f32)
            nc.sync.dma_start(segcol_sb[:], segcol_in[:])
            wblk_sb = constp.tile([L, 1], f32)
            nc.sync.dma_start(wblk_sb[:], wblk_in[:])
            crecip_sb = constp.tile([L, 1], f32)
            nc.sync.dma_start(crecip_sb[:], crecip_in[:])
            perms_sb = constp.tile([L, (L - 1) * L], f16)
            nc.sync.dma_start(perms_sb[:], perms_in[:])
            negdv_sb = constp.tile([128, 1], f32)
            nc.vector.memset(negdv_sb[:], -DELTA_V)
            eps_sb = constp.tile([128, 1], f32)
            nc.vector.memset(eps_sb[:], 1e-8)

            # ---- resident x (chunked so reads can start early) ----
            xch = []
            for i in range(NCH):
                c0, c1 = i * CHT, min((i + 1) * CHT, T_pad)
                xt_ch = xp.tile([128, (c1 - c0) * D], f16, tag="xch",
                                name=f"xch{i}")
                nc.sync.dma_start(xt_ch[:], x_in.ap()[:, c0 * D:c1 * D])
                xch.append(xt_ch)

            def xt(t):
                return xch[t // CHT][:, (t % CHT) * D:(t % CHT + 1) * D]

            for rep in range(reps):
                ss_all = normp.tile([128, T_pad], f32, tag="ss", name="ss")
                rr_all = normp.tile([128, T_pad], f32, tag="rr", name="rr")
                d1_all = normp.tile([128, T_pad], f32, tag="d1", name="d1")
                wc_all = normp.tile([128, T_pad], f32, tag="wc", name="wc")

                # ---- pass 1: local centroid sums of normalized points ----
                with tc.tile_pool(name="sumsps", bufs=1, space="PSUM") as sp:
                    ps_sums = sp.tile([L, D], f32, tag="sums", name="ps_sums")
                    for g in range(T_pad // G if "p1" in phases else 0):
                        t0 = g * G
                        for j in range(G):
                            t = t0 + j
                            sq_sink = sinkp.tile([128, D], f16, tag="sqsink")
                            nc.scalar.activation(
                                sq_sink[:], xt(t), ActFn.Square,
                                accum_out=ss_all[:, t:t + 1],
                            )
                        nn_g = grpp.tile([128, G], f32, tag="nn")
                        nc.scalar.activation(
                            nn_g[:], ss_all[:, t0:t0 + G],
                            ActFn.Sqrt, bias=eps_sb[:],
                        )
                        nc.vector.reciprocal(rr_all[:, t0:t0 + G], nn_g[:])
                        for j in range(G):
                            t = t0 + j
                            oh = ohp.tile([128, L], f16, tag="oh")
                            nc.vector.tensor_scalar(
                                oh[:], iota64_sb[:], segcol_sb[:, t:t + 1],
                                rr_all[:, t:t + 1],
                                op0=AluOp.is_equal, op1=AluOp.mult,
                            )
                            nc.tensor.matmul(
                                ps_sums[:], oh[:], xt(t),
                                start=(t == 0), stop=(t == T_pad - 1),
                            )

                    # ---- mu table: [128, 257] f16, rows 64.. zero ----
                    mut_h = mutp.tile([128, D + 1], f16, tag="mut",
                                      name="mut_h")
                    nc.vector.memset(mut_h[:], 0.0)
                    if "p1" in phases:
                        nc.vector.tensor_scalar(
                            mut_h[0:L, 0:D], ps_sums[:], crecip_sb[:], None,
                            op0=AluOp.mult,
                        )
                    nc.vector.tensor_copy(mut_h[0:L, D:D + 1], wblk_sb[:])

                # ---- push: rotation distances on own mu block ----
                q_sb = smallp.tile([L, L], f32, tag="q", name="q_sb")
                nc.vector.memset(q_sb[:, 0:1], 0.0)
                with (
                    tc.tile_pool(name="rotps", bufs=2, space="PSUM") as rotp,
                    tc.tile_pool(name="pdiff", bufs=3) as pdp,
                ):
                    for k in range(1, L if "push" in phases else 1):
                        ps_rot = rotp.tile([L, D], f32, tag="rot")
                        nc.tensor.matmul(
                            ps_rot[:], perms_sb[:, (k - 1) * L:k * L],
                            mut_h[0:L, 0:D], start=True, stop=True,
                        )
                        pdiff = pdp.tile([L, D], f32, tag="pdiff")
                        nc.vector.tensor_sub(pdiff[:], mut_h[0:L, 0:D],
                                             ps_rot[:])
                        psink = pdp.tile([L, D], f32, tag="psink")
                        nc.scalar.activation(
                            psink[:], pdiff[:], ActFn.Abs,
                            accum_out=q_sb[:, k:k + 1],
                        )
                nc.sync.dma_start(q_out[:], q_sb[:])

                # ---- pass 2: pull term ----
                with (
                    tc.tile_pool(name="bcps", bufs=3, space="PSUM") as bcp,
                    tc.tile_pool(name="mups", bufs=3, space="PSUM") as mup,
                    tc.tile_pool(name="tsps", bufs=1, space="PSUM") as tsp,
                ):
                    ps_ts = tsp.tile([1, T_pad], f32, tag="ts", name="ps_ts")
                    if "p2" not in phases:
                        nc.vector.memset(ps_ts[:], 0.0)
                    for g in range(T_pad // G if "p2" in phases else 0):
                        t0 = g * G
                        srow_g = srowp.tile([1, G * 128], f16, tag="srow")
                        nc.sync.dma_start(
                            srow_g[:],
                            segrow_in.ap()[t0 * 128:(t0 + G) * 128]
                            .rearrange("(a i) -> a i", a=1),
                        )
                        for j in range(G):
                            t = t0 + j
                            ps_bc = bcp.tile([128, 128], f32, tag="bc")
                            nc.tensor.matmul(
                                ps_bc[:], ones_sb[:],
                                srow_g[:, j * 128:(j + 1) * 128],
                                start=True, stop=True,
                            )
                            oht = ohtp.tile([128, 128], f16, tag="oht")
                            nc.vector.tensor_scalar(
                                oht[:], ps_bc[:], iotap_sb[:], None,
                                op0=AluOp.is_equal,
                            )
                            ps_mu = mup.tile([128, D + 1], f32, tag="mu")
                            nc.tensor.matmul(
                                ps_mu[:], oht[:], mut_h[:],
                                start=True, stop=True,
                            )
                            diff = diffp.tile([128, D], f32, tag="diff")
                            nc.vector.scalar_tensor_tensor(
                                diff[:], xt(t), rr_all[:, t:t + 1],
                                ps_mu[:, 0:D],
                                op0=AluOp.mult, op1=AluOp.subtract,
                            )
                            ab_sink = sinkp.tile([128, D], f32, tag="absink")
                            nc.scalar.activation(
                                ab_sink[:], diff[:], ActFn.Abs,
                                accum_out=d1_all[:, t:t + 1],
                            )
                            nc.vector.tensor_copy(
                                wc_all[:, t:t + 1], ps_mu[:, D:D + 1],
                            )
                        t1g = grpp.tile([128, G], f32, tag="t1g")
                        nc.scalar.activation(
                            t1g[:], d1_all[:, t0:t0 + G], ActFn.Relu,
                            bias=negdv_sb[:],
                        )
                        t2g = grpp.tile([128, G], f32, tag="t2g")
                        nc.vector.tensor_mul(t2g[:], t1g[:], t1g[:])
                        vg = grpp.tile([128, G], f32, tag="vg")
                        nc.vector.tensor_mul(vg[:], t2g[:],
                                             wc_all[:, t0:t0 + G])
                        nc.tensor.matmul(
                            ps_ts[0:1, t0:t0 + G], onesw_sb[:], vg[:],
                            start=True, stop=True,
                        )
                    ts_sb = smallp.tile([1, T_pad], f32, tag="tssb")
                    nc.vector.tensor_copy(ts_sb[:], ps_ts[:])
                    nc.sync.dma_start(ts_out[:], ts_sb[:])

    nc.compile()
    return nc


def host_tables(labels: np.ndarray, subbatch: np.ndarray):
    """Everything derivable from the integer inputs alone."""
    seg = (subbatch.astype(np.int64) * L + labels.astype(np.int64)).astype(np.int32)
    counts = np.bincount(seg, minlength=S * L).astype(np.float64)
    present = counts > 0
    M = present.reshape(S, L).sum(axis=1).astype(np.float64)
    valid = M > 1.0
    M_per_seg = np.repeat(M, L)
    valid_per_seg = np.repeat(valid, L)
    w = np.where(
        valid_per_seg, 1.0 / (M_per_seg * np.maximum(counts, 1.0)), 0.0
    ).astype(np.float32)
    crecip = (1.0 / np.maximum(counts, 1.0)).astype(np.float32)
    return seg, counts, present, M, valid, w, crecip


def pick_tpad(subbatch: np.ndarray) -> int:
    counts_sb = np.bincount(subbatch, minlength=S)
    T = int(np.ceil(counts_sb.max() / 128))
    return ((T + G - 1) // G) * G


def make_in_maps(outputs: np.ndarray, labels: np.ndarray,
                 subbatch: np.ndarray, T_pad: int | None = None):
    n = outputs.shape[0]
    tables = host_tables(labels, subbatch)
    seg, counts, present, M, valid, w, crecip = tables
    if T_pad is None:
        T_pad = pick_tpad(subbatch)
    n_pad = T_pad * 128

    order = np.argsort(subbatch, kind="stable")
    counts_sb = np.bincount(subbatch, minlength=S)
    offs = np.concatenate([[0], np.cumsum(counts_sb)])

    xh = outputs.astype(np.float16)
    labf = labels.astype(np.float32)

    iota64 = np.broadcast_to(
        np.arange(L, dtype=np.float16), (128, L)).copy()
    iotap = np.arange(128, dtype=np.float32).reshape(128, 1)
    ones1 = np.ones((1, 128), np.float16)
    onesw = np.ones((128, 1), np.float32)
    pp, kk, mm = np.meshgrid(
        np.arange(L), np.arange(1, L), np.arange(L), indexing="ij")
    perms = np.ascontiguousarray(
        (pp == (mm + kk) % L).astype(np.float16).reshape(L, (L - 1) * L))

    in_maps = []
    for c in range(NCORES):
        idx = order[offs[c]:offs[c + 1]]
        cnt = idx.size
        xc = np.zeros((n_pad, D), np.float16)
        xc[:cnt] = xh[idx]
        xc = np.ascontiguousarray(
            xc.reshape(T_pad, 128, D).transpose(1, 0, 2)
        ).reshape(128, T_pad * D)
        segl = np.full((n_pad,), -1.0, np.float32)
        segl[:cnt] = labf[idx]
        blk = slice(c * L, (c + 1) * L)
        in_maps.append({
            "x": xc,
            "segrow": segl.astype(np.float16),
            "segcol": np.ascontiguousarray(segl.reshape(T_pad, 128).T),
            "iota64": iota64,
            "iotap": iotap,
            "ones1": ones1,
            "onesw": onesw,
            "perms": perms,
            "wblk": (w[blk] * WSCALE).reshape(L, 1).astype(np.float32),
            "crecip": crecip[blk].reshape(L, 1).astype(np.float32),
        })
    return in_maps, tables, T_pad


def combine(results, tables, n: int):
    """Host combine of the per-core outputs into the scalar loss."""
    seg, counts, present, M, valid, w, crecip = tables
    pull_total = np.float64(0.0)
    for r in results:
        pull_total += r["ts"].astype(np.float64).sum() / WSCALE

    push_total = np.float64(0.0)
    pres_sl = present.reshape(S, L)
    for sb in range(S):
        if not valid[sb]:
            continue
        q = results[sb]["q"].astype(np.float64)  # [64(a), 64(k)]
        a = np.arange(L)
        dist = np.zeros((L, L))
        for k in range(1, L):
            dist[a, (a + k) % L] = q[:, k]
        p = pres_sl[sb]
        mask = p[:, None] & p[None, :] & ~np.eye(L, dtype=bool)
        r = np.maximum(2.0 * DELTA_D - dist, 0.0) ** 2
        push = np.where(mask, r, 0.0).sum()
        push_total += push / max(M[sb] * (M[sb] - 1.0), 1.0)

    return np.float32((pull_total + push_total) / n)


_NC_CACHE: dict = {}


def _get_nc(T_pad: int):
    if T_pad not in _NC_CACHE:
        _NC_CACHE[T_pad] = build_nc(T_pad)
    return _NC_CACHE[T_pad]


def kernel(outputs, labels, subbatch_indices):
    from concourse.bass_utils import run_bass_kernel_spmd

    outputs = np.asarray(outputs, dtype=np.float32)
    labels = np.asarray(labels, dtype=np.int32)
    subbatch_indices = np.asarray(subbatch_indices, dtype=np.int32)
    n = outputs.shape[0]

    in_maps, tables, T_pad = make_in_maps(outputs, labels, subbatch_indices)
    nc = _get_nc(T_pad)
    res = run_bass_kernel_spmd(nc, in_maps, list(range(NCORES)))
    return combine(res.results, tables, n)


# revision 7
# speedup vs baseline: 8.8271x; 1.6911x over previous
"""CentroidInstanceLoss on 8 Trainium2 NeuronCores (Bass/Tile).

Subbatch-parallel: core c processes exactly the points of subbatch c
(S=8 == NCORES), padded to a common tile count T_pad. All centroid
segment-sums are then core-LOCAL (64 labels per core): no collectives,
no cross-core barrier. Host does only integer bookkeeping (counts, M,
pull weights, the subbatch partition/pad permutation) and the final
O(S*L) combine.

Per core:
  pass 1: x resident in SBUF; per 128-point tile: sum-of-squares accum
          (ACT), rsqrt per group, one-hot [128,64]*rr (DVE), one matmul
          accumulating the [64, 256] centroid sums in PSUM.
  mu:     scale by 1/counts, append w*1024 column -> [128, 257] f16
          table (rows 64..127 zero).
  push:   63 partition-rotation matmuls on the [64,256] mu block.
  pass 2: per tile: seg broadcast matmul, one-hot transpose compare
          (DVE, from PSUM), one gather matmul -> per-point mu+w, diff,
          L1 accum; per group of G tiles: relu/square/weight + a
          ones-matmul partition reduction into per-tile pull sums.
Outputs: q [64,64] rotation distances, ts [1, T_pad] weighted pull
sums. Host: loss = (sum(ts)/WSCALE + push)/N.
"""

import numpy as np

import concourse.bass as bass
import concourse.bacc as bacc
import concourse.mybir as mybir
import concourse.tile as tile

f32 = mybir.dt.float32
f16 = mybir.dt.float16

# Problem shape (hardcoded per contract).
N_TOTAL = 262144
D = 256
S = 8
L = 64
NCORES = 8
DELTA_V = 0.5
DELTA_D = 1.5
G = 16          # tiles per group
WSCALE = 1024.0  # pull-weight scale to keep w in f16 normal range

AluOp = mybir.AluOpType
ActFn = mybir.ActivationFunctionType


def build_nc(T_pad: int, reps: int = 1,
             phases: tuple = ("p1", "push", "p2")):
    """SPMD program for one core holding T_pad tiles of 128 points."""
    assert T_pad % G == 0
    NCH = 16                     # x load chunks
    CHT = (T_pad + NCH - 1) // NCH
    BC = 4                       # tiles per seg-broadcast matmul

    nc = bacc.Bacc("TRN2", target_bir_lowering=False, debug=False,
                   num_devices=1)

    x_in = nc.dram_tensor("x", [128, T_pad * D], f16, kind="ExternalInput")
    segrow_in = nc.dram_tensor("segrow", [T_pad * 128], f16,
                               kind="ExternalInput")
    segcol_in = nc.dram_tensor("segcol", [128, T_pad], f32,
                               kind="ExternalInput")
    iota64_in = nc.dram_tensor("iota64", [128, L], f16, kind="ExternalInput")
    iotap_in = nc.dram_tensor("iotap", [128, 1], f32, kind="ExternalInput")
    ones_in = nc.dram_tensor("ones1", [1, 128], f16, kind="ExternalInput")
    onesw_in = nc.dram_tensor("onesw", [128, 1], f32, kind="ExternalInput")
    perms_in = nc.dram_tensor("perms", [L, (L - 1) * L], f16,
                              kind="ExternalInput")
    wblk_in = nc.dram_tensor("wblk", [L, 1], f32, kind="ExternalInput")
    crecip_in = nc.dram_tensor("crecip", [L, 1], f32, kind="ExternalInput")

    q_out = nc.dram_tensor("q", [L, L], f32, kind="ExternalOutput")
    ts_out = nc.dram_tensor("ts", [1, T_pad], f32, kind="ExternalOutput")

    with tile.TileContext(nc) as tc:
        with (
            tc.tile_pool(name="const", bufs=1) as constp,
            tc.tile_pool(name="xres", bufs=NCH) as xp,
            tc.tile_pool(name="norm", bufs=1) as normp,
            tc.tile_pool(name="oh", bufs=4) as ohp,
            tc.tile_pool(name="srow", bufs=3) as srowp,
            tc.tile_pool(name="oht", bufs=8) as ohtp,
            tc.tile_pool(name="diff", bufs=4) as diffp,
            tc.tile_pool(name="sink", bufs=3) as sinkp,
            tc.tile_pool(name="mut", bufs=1) as mutp,
            tc.tile_pool(name="grp", bufs=2) as grpp,
            tc.tile_pool(name="small", bufs=2) as smallp,
        ):
            # ---- constants ----
            iota64_sb = constp.tile([128, L], f16)
            nc.sync.dma_start(iota64_sb[:], iota64_in[:])
            iotap_sb = constp.tile([128, 1], f32)
            nc.sync.dma_start(iotap_sb[:], iotap_in[:])
            ones_sb = constp.tile([1, 128], f16)
            nc.sync.dma_start(ones_sb[:], ones_in[:])
            onesw_sb = constp.tile([128, 1], f32)
            nc.sync.dma_start(onesw_sb[:], onesw_in[:])
            segcol_sb = constp.tile([128, T_pad], f32)
            nc.sync.dma_start(segcol_sb[:], segcol_in[:])
            wblk_sb = constp.tile([L, 1], f32)
            nc.sync.dma_start(wblk_sb[:], wblk_in[:])
            crecip_sb = constp.tile([L, 1], f32)
            nc.sync.dma_start(crecip_sb[:], crecip_in[:])
            perms_sb = constp.tile([L, (L - 1) * L], f16)
            nc.sync.dma_start(perms_sb[:], perms_in[:])
            negdv_sb = constp.tile([128, 1], f32)
            nc.vector.memset(negdv_sb[:], -DELTA_V)
            eps_sb = constp.tile([128, 1], f32)
            nc.vector.memset(eps_sb[:], 1e-8)

            # ---- resident x (chunked so reads can start early) ----
            xch = []
            for i in range(NCH):
                c0, c1 = i * CHT, min((i + 1) * CHT, T_pad)
                xt_ch = xp.tile([128, (c1 - c0) * D], f16, tag="xch",
                                name=f"xch{i}")
                nc.sync.dma_start(xt_ch[:], x_in.ap()[:, c0 * D:c1 * D])
                xch.append(xt_ch)

            def xt(t):
                return xch[t // CHT][:, (t % CHT) * D:(t % CHT + 1) * D]

            for rep in range(reps):
                ss_all = normp.tile([128, T_pad], f32, tag="ss", name="ss")
                rr_all = normp.tile([128, T_pad], f32, tag="rr", name="rr")
                d1_all = normp.tile([128, T_pad], f32, tag="d1", name="d1")
                wc_all = normp.tile([128, T_pad], f32, tag="wc", name="wc")

                # ---- pass 1: local centroid sums of normalized points ----
                with tc.tile_pool(name="sumsps", bufs=1, space="PSUM") as sp:
                    ps_sums = sp.tile([L, D], f32, tag="sums", name="ps_sums")
                    for g in range(T_pad // G if "p1" in phases else 0):
                        t0 = g * G
                        for j in range(G):
                            t = t0 + j
                            sq_sink = sinkp.tile([128, D], f16, tag="sqsink")
                            nc.scalar.activation(
                                sq_sink[:], xt(t), ActFn.Square,
                                accum_out=ss_all[:, t:t + 1],
                            )
                        nn_g = grpp.tile([128, G], f32, tag="nn")
                        nc.scalar.activation(
                            nn_g[:], ss_all[:, t0:t0 + G],
                            ActFn.Sqrt, bias=eps_sb[:],
                        )
                        nc.vector.reciprocal(rr_all[:, t0:t0 + G], nn_g[:])
                        for j in range(G):
                            t = t0 + j
                            oh = ohp.tile([128, L], f16, tag="oh")
                            nc.vector.tensor_scalar(
                                oh[:], iota64_sb[:], segcol_sb[:, t:t + 1],
                                rr_all[:, t:t + 1],
                                op0=AluOp.is_equal, op1=AluOp.mult,
                            )
                            nc.tensor.matmul(
                                ps_sums[:], oh[:], xt(t),
                                start=(t == 0), stop=(t == T_pad - 1),
                            )

                    # ---- mu table: [128, 257] f16, rows 64.. zero ----
                    mut_h = mutp.tile([128, D + 1], f16, tag="mut",
                                      name="mut_h")
                    nc.vector.memset(mut_h[:], 0.0)
                    if "p1" in phases:
                        nc.vector.tensor_scalar(
                            mut_h[0:L, 0:D], ps_sums[:], crecip_sb[:], None,
                            op0=AluOp.mult,
                        )
                    nc.vector.tensor_copy(mut_h[0:L, D:D + 1], wblk_sb[:])

                # ---- push: rotation distances on own mu block ----
                q_sb = smallp.tile([L, L], f32, tag="q", name="q_sb")
                nc.vector.memset(q_sb[:, 0:1], 0.0)
                with (
                    tc.tile_pool(name="rotps", bufs=2, space="PSUM") as rotp,
                    tc.tile_pool(name="pdiff", bufs=3) as pdp,
                ):
                    for k in range(1, L if "push" in phases else 1):
                        ps_rot = rotp.tile([L, D], f32, tag="rot")
                        nc.tensor.matmul(
                            ps_rot[:], perms_sb[:, (k - 1) * L:k * L],
                            mut_h[0:L, 0:D], start=True, stop=True,
                        )
                        pdiff = pdp.tile([L, D], f32, tag="pdiff")
                        nc.vector.tensor_sub(pdiff[:], mut_h[0:L, 0:D],
                                             ps_rot[:])
                        psink = pdp.tile([L, D], f32, tag="psink")
                        nc.scalar.activation(
                            psink[:], pdiff[:], ActFn.Abs,
                            accum_out=q_sb[:, k:k + 1],
                        )
                nc.sync.dma_start(q_out[:], q_sb[:])

                # ---- pass 2: pull term ----
                with (
                    tc.tile_pool(name="bcps", bufs=2, space="PSUM") as bcp,
                    tc.tile_pool(name="mups", bufs=3, space="PSUM") as mup,
                    tc.tile_pool(name="tsps", bufs=1, space="PSUM") as tsp,
                ):
                    ps_ts = tsp.tile([1, T_pad], f32, tag="ts", name="ps_ts")
                    if "p2" not in phases:
                        nc.vector.memset(ps_ts[:], 0.0)
                    for g in range(T_pad // G if "p2" in phases else 0):
                        t0 = g * G
                        srow_g = srowp.tile([1, G * 128], f16, tag="srow")
                        nc.sync.dma_start(
                            srow_g[:],
                            segrow_in.ap()[t0 * 128:(t0 + G) * 128]
                            .rearrange("(a i) -> a i", a=1),
                        )
                        oht4s = []
                        for b in range(G // BC):
                            ps_bc = bcp.tile([128, BC * 128], f32, tag="bc")
                            nc.tensor.matmul(
                                ps_bc[:], ones_sb[:],
                                srow_g[:, b * BC * 128:(b + 1) * BC * 128],
                                start=True, stop=True,
                            )
                            oht4 = ohtp.tile([128, BC * 128], f16, tag="oht")
                            nc.vector.tensor_scalar(
                                oht4[:], ps_bc[:], iotap_sb[:], None,
                                op0=AluOp.is_equal,
                            )
                            oht4s.append(oht4)
                        for j in range(G):
                            t = t0 + j
                            oht = oht4s[j // BC][:, (j % BC) * 128:
                                                 (j % BC + 1) * 128]
                            ps_mu = mup.tile([128, D + 1], f32, tag="mu")
                            nc.tensor.matmul(
                                ps_mu[:], oht, mut_h[:],
                                start=True, stop=True,
                            )
                            diff = diffp.tile([128, D], f32, tag="diff")
                            nc.vector.scalar_tensor_tensor(
                                diff[:], xt(t), rr_all[:, t:t + 1],
                                ps_mu[:, 0:D],
                                op0=AluOp.mult, op1=AluOp.subtract,
                            )
                            ab_sink = sinkp.tile([128, D], f32, tag="absink")
                            nc.scalar.activation(
                                ab_sink[:], diff[:], ActFn.Abs,
                                accum_out=d1_all[:, t:t + 1],
                            )
                            nc.vector.tensor_copy(
                                wc_all[:, t:t + 1], ps_mu[:, D:D + 1],
                            )
                        t1g = grpp.tile([128, G], f32, tag="t1g")
                        nc.scalar.activation(
                            t1g[:], d1_all[:, t0:t0 + G], ActFn.Relu,
                            bias=negdv_sb[:],
                        )
                        t2g = grpp.tile([128, G], f32, tag="t2g")
                        nc.vector.tensor_mul(t2g[:], t1g[:], t1g[:])
                        vg = grpp.tile([128, G], f32, tag="vg")
                        nc.vector.tensor_mul(vg[:], t2g[:],
                                             wc_all[:, t0:t0 + G])
                        nc.tensor.matmul(
                            ps_ts[0:1, t0:t0 + G], onesw_sb[:], vg[:],
                            start=True, stop=True,
                        )
                    ts_sb = smallp.tile([1, T_pad], f32, tag="tssb")
                    nc.vector.tensor_copy(ts_sb[:], ps_ts[:])
                    nc.sync.dma_start(ts_out[:], ts_sb[:])

    nc.compile()
    return nc


def host_tables(labels: np.ndarray, subbatch: np.ndarray):
    """Everything derivable from the integer inputs alone."""
    seg = (subbatch.astype(np.int64) * L + labels.astype(np.int64)).astype(np.int32)
    counts = np.bincount(seg, minlength=S * L).astype(np.float64)
    present = counts > 0
    M = present.reshape(S, L).sum(axis=1).astype(np.float64)
    valid = M > 1.0
    M_per_seg = np.repeat(M, L)
    valid_per_seg = np.repeat(valid, L)
    w = np.where(
        valid_per_seg, 1.0 / (M_per_seg * np.maximum(counts, 1.0)), 0.0
    ).astype(np.float32)
    crecip = (1.0 / np.maximum(counts, 1.0)).astype(np.float32)
    return seg, counts, present, M, valid, w, crecip


def pick_tpad(subbatch: np.ndarray) -> int:
    counts_sb = np.bincount(subbatch, minlength=S)
    T = int(np.ceil(counts_sb.max() / 128))
    return ((T + G - 1) // G) * G


def make_in_maps(outputs: np.ndarray, labels: np.ndarray,
                 subbatch: np.ndarray, T_pad: int | None = None):
    n = outputs.shape[0]
    tables = host_tables(labels, subbatch)
    seg, counts, present, M, valid, w, crecip = tables
    if T_pad is None:
        T_pad = pick_tpad(subbatch)
    n_pad = T_pad * 128

    order = np.argsort(subbatch, kind="stable")
    counts_sb = np.bincount(subbatch, minlength=S)
    offs = np.concatenate([[0], np.cumsum(counts_sb)])

    xh = outputs.astype(np.float16)
    labf = labels.astype(np.float32)

    iota64 = np.broadcast_to(
        np.arange(L, dtype=np.float16), (128, L)).copy()
    iotap = np.arange(128, dtype=np.float32).reshape(128, 1)
    ones1 = np.ones((1, 128), np.float16)
    onesw = np.ones((128, 1), np.float32)
    pp, kk, mm = np.meshgrid(
        np.arange(L), np.arange(1, L), np.arange(L), indexing="ij")
    perms = np.ascontiguousarray(
        (pp == (mm + kk) % L).astype(np.float16).reshape(L, (L - 1) * L))

    in_maps = []
    for c in range(NCORES):
        idx = order[offs[c]:offs[c + 1]]
        cnt = idx.size
        xc = np.zeros((n_pad, D), np.float16)
        xc[:cnt] = xh[idx]
        xc = np.ascontiguousarray(
            xc.reshape(T_pad, 128, D).transpose(1, 0, 2)
        ).reshape(128, T_pad * D)
        segl = np.full((n_pad,), -1.0, np.float32)
        segl[:cnt] = labf[idx]
        blk = slice(c * L, (c + 1) * L)
        in_maps.append({
            "x": xc,
            "segrow": segl.astype(np.float16),
            "segcol": np.ascontiguousarray(segl.reshape(T_pad, 128).T),
            "iota64": iota64,
            "iotap": iotap,
            "ones1": ones1,
            "onesw": onesw,
            "perms": perms,
            "wblk": (w[blk] * WSCALE).reshape(L, 1).astype(np.float32),
            "crecip": crecip[blk].reshape(L, 1).astype(np.float32),
        })
    return in_maps, tables, T_pad


def combine(results, tables, n: int):
    """Host combine of the per-core outputs into the scalar loss."""
    seg, counts, present, M, valid, w, crecip = tables
    pull_total = np.float64(0.0)
    for r in results:
        pull_total += r["ts"].astype(np.float64).sum() / WSCALE

    push_total = np.float64(0.0)
    pres_sl = present.reshape(S, L)
    for sb in range(S):
        if not valid[sb]:
            continue
        q = results[sb]["q"].astype(np.float64)  # [64(a), 64(k)]
        a = np.arange(L)
        dist = np.zeros((L, L))
        for k in range(1, L):
            dist[a, (a + k) % L] = q[:, k]
        p = pres_sl[sb]
        mask = p[:, None] & p[None, :] & ~np.eye(L, dtype=bool)
        r = np.maximum(2.0 * DELTA_D - dist, 0.0) ** 2
        push = np.where(mask, r, 0.0).sum()
        push_total += push / max(M[sb] * (M[sb] - 1.0), 1.0)

    return np.float32((pull_total + push_total) / n)


_NC_CACHE: dict = {}


def _get_nc(T_pad: int):
    if T_pad not in _NC_CACHE:
        _NC_CACHE[T_pad] = build_nc(T_pad)
    return _NC_CACHE[T_pad]


def kernel(outputs, labels, subbatch_indices):
    from concourse.bass_utils import run_bass_kernel_spmd

    outputs = np.asarray(outputs, dtype=np.float32)
    labels = np.asarray(labels, dtype=np.int32)
    subbatch_indices = np.asarray(subbatch_indices, dtype=np.int32)
    n = outputs.shape[0]

    in_maps, tables, T_pad = make_in_maps(outputs, labels, subbatch_indices)
    nc = _get_nc(T_pad)
    res = run_bass_kernel_spmd(nc, in_maps, list(range(NCORES)))
    return combine(res.results, tables, n)
